# revision 5
# baseline (speedup 1.0000x reference)
"""KAN Fourier-linear kernel for 8 Trainium2 NeuronCores.

y[n,o] = sum_{i,g} C0[o,i,g]*cos(g*x[n,i]) + C1[o,i,g]*sin(g*x[n,i]) + bias[o]

Strategy (data-parallel over n, 4096 rows/core), double-angle cascade:
  - ACT (ScalarE) computes sin/cos only for odd g in {1,3,...,15} (16 Sin
    passes per superpass instead of 64), via the int-round range reduction:
      v   = int32(x*g/2pi + magic)        # gpsimd tensor_scalar
      r_g = x - v*(2pi/g)                 # DVE scalar_tensor_tensor (fp32)
      s_g = Sin(scale=g, bias=b_s)(r_g); c_g = Sin(scale=g, bias=b_c)(r_g)
  - Even harmonics come from 1-op DVE fp16 products with the scale factors
    folded into the weights host-side:
      u_m = s_m*c_m   covers sin(2m x) = kappa_m * u_m
      v_m = s_m*s_m   covers cos(2m x) = 1 - lambda_m * v_m
    (constants fold into the output bias). True-cos intermediates C_{2m} =
    1 - lambda*v_m (one tensor_scalar) extend the cascade to m in {2,4,6,8}.
  - Everything in fp16 (not bf16): the x128 weight folds amplify feature
    rounding error; fp16's 10-bit mantissa keeps rel err ~2e-3.
  - y.T tile = W.T @ F via PE, K=4096 accumulated in PSUM (fp16 inputs).
  - PSUM evicted by ACT Copy with fused per-partition bias add, fp16 out.
"""
import math
import numpy as np
from contextlib import ExitStack

import concourse.bass as bass
import concourse.mybir as mybir
import concourse.tile as tile
from concourse import bacc
from concourse.bass_utils import run_bass_kernel_spmd

N_CORES = 8
N_TOTAL = 32768
N_SHARD = N_TOTAL // N_CORES        # 4096 rows per core
INDIM = 128
OUTDIM = 256
GRID = 16
K_TOT = 2 * GRID * INDIM            # 4096
SP = 2                              # n-superpasses per core
S = N_SHARD // SP                   # 2048 columns per superpass
CH = 512                            # matmul moving chunk (ISA max)
TWO_PI = 2.0 * math.pi

FP32 = mybir.dt.float32
FP16 = mybir.dt.float16
I32 = mybir.dt.int32

ODD = (1, 3, 5, 7, 9, 11, 13, 15)
# kt consumption order: feature name per contraction block.
# Interleaved so production (ACT for s/c, DVE for u/v) stays ahead of the PE.
KT_ORDER = [
    ("s", 1), ("c", 1), ("u", 1), ("v", 1),
    ("s", 3), ("c", 3), ("u", 3), ("v", 3),
    ("s", 5), ("c", 5), ("u", 5), ("v", 5),
    ("s", 7), ("c", 7), ("u", 7), ("v", 7),
    ("s", 9), ("c", 9), ("u", 2), ("v", 2),
    ("s", 11), ("c", 11), ("u", 4), ("v", 4),
    ("s", 13), ("c", 13), ("u", 6), ("v", 6),
    ("s", 15), ("c", 15), ("u", 8), ("v", 8),
]
KAPPA = {1: 2.0, 2: 4.0, 3: 2.0, 4: 8.0, 5: 2.0, 6: 4.0, 7: 2.0, 8: 16.0}
LAMBDA = {1: 2.0, 2: 8.0, 3: 2.0, 4: 32.0, 5: 2.0, 6: 8.0, 7: 2.0, 8: 128.0}


def _g_consts(g: int):
    a = np.float32(g / TWO_PI)
    phat = np.float32(TWO_PI / g)
    m = 2.0 ** math.ceil(math.log2(0.960 * g + 0.14))
    c = np.float32(m + 0.125)
    b_s = np.float32(m * g * float(phat))      # == 2pi*m up to fp32, matched to phat
    b_c = np.float32(float(b_s) + math.pi / 2.0)
    return a, phat, c, b_s, b_c


_CACHED = {}


def _build(reps: int = 1):
    key = ("nc", reps)
    if key in _CACHED:
        return _CACHED[key]
    nc = bacc.Bacc("TRN2", target_bir_lowering=False, debug=False,
                   num_devices=N_CORES)
    xt_d = nc.dram_tensor("xt", [INDIM, N_SHARD], FP32, kind="ExternalInput").ap()
    w_d = nc.dram_tensor("w", [INDIM, 32 * OUTDIM], FP16, kind="ExternalInput").ap()
    bt_d = nc.dram_tensor("bt", [INDIM, 16], FP32, kind="ExternalInput").ap()
    bias_d = nc.dram_tensor("bias", [INDIM, 2], FP32, kind="ExternalInput").ap()
    yt_d = nc.dram_tensor("yt", [OUTDIM, N_SHARD], FP16, kind="ExternalOutput").ap()

    with tile.TileContext(nc) as tc, ExitStack() as ctx:
        cpool = ctx.enter_context(tc.tile_pool(name="const", bufs=1))
        vpool = ctx.enter_context(tc.tile_pool(name="v", bufs=2))
        rpool = ctx.enter_context(tc.tile_pool(name="r", bufs=2))
        apool = ctx.enter_context(tc.tile_pool(name="af", bufs=6))
        dpool = ctx.enter_context(tc.tile_pool(name="df", bufs=1))
        ypool = ctx.enter_context(tc.tile_pool(name="y", bufs=2))
        ppool = ctx.enter_context(tc.tile_pool(name="psum", bufs=1, space="PSUM"))

        xt = cpool.tile([INDIM, N_SHARD], FP32)
        nc.sync.dma_start(xt[:], xt_d[:])
        wt = cpool.tile([INDIM, 32 * OUTDIM], FP16)
        nc.sync.dma_start(wt[:], w_d[:])
        bt = cpool.tile([INDIM, 16], FP32)
        nc.sync.dma_start(bt[:], bt_d[:])
        bias = cpool.tile([INDIM, 2], FP32)
        nc.sync.dma_start(bias[:], bias_d[:])

        def emit_evict(psums, sp):
            for oh in range(2):
                y = ypool.tile([128, S], FP16, tag=f"y{oh}")
                nc.scalar.activation(y[:], psums[oh][:],
                                     mybir.ActivationFunctionType.Identity,
                                     bias=bias[:, oh:oh + 1], scale=1.0)
                nc.sync.dma_start(
                    yt_d[oh * 128:(oh + 1) * 128, sp * S:(sp + 1) * S], y[:])

        def body():
            pending = None
            for sp in range(SP):
                xs = xt[:, sp * S:(sp + 1) * S]
                feats = {}

                # -- range reduction + ACT passes for odd g --
                def reduce_g(g, j):
                    a, phat, c, b_s, b_c = _g_consts(g)
                    v = vpool.tile([INDIM, S], I32, tag="v")
                    nc.gpsimd.tensor_scalar(v[:], xs, float(a), float(c),
                                            mybir.AluOpType.mult,
                                            mybir.AluOpType.add)
                    r = rpool.tile([INDIM, S], FP32, tag="r")
                    nc.vector.scalar_tensor_tensor(r[:], v[:], float(-phat), xs,
                                                   mybir.AluOpType.mult,
                                                   mybir.AluOpType.add)
                    s = apool.tile([INDIM, S], FP16, tag="act")
                    nc.scalar.activation(s[:], r[:],
                                         mybir.ActivationFunctionType.Sin,
                                         bias=bt[:, 2 * j:2 * j + 1],
                                         scale=float(g))
                    cc = apool.tile([INDIM, S], FP16, tag="act")
                    nc.scalar.activation(cc[:], r[:],
                                         mybir.ActivationFunctionType.Sin,
                                         bias=bt[:, 2 * j + 1:2 * j + 2],
                                         scale=float(g))
                    feats[("s", g)] = s
                    feats[("c", g)] = cc

                def prod(name, m, in0, in1):
                    t = dpool.tile([INDIM, S], FP16, tag=f"{name}{m}")
                    nc.vector.tensor_tensor(t[:], in0[:], in1[:],
                                            mybir.AluOpType.mult)
                    feats[(name, m)] = t
                    return t

                def truecos(m, vm, lam):
                    t = dpool.tile([INDIM, S], FP16, tag=f"C{2*m}")
                    nc.vector.tensor_scalar(t[:], vm[:], float(-lam), 1.0,
                                            mybir.AluOpType.mult,
                                            mybir.AluOpType.add)
                    return t

                # emission order = per-engine program order; keep DVE stream
                # interleaved so fmas feed ACT early and cascade flows.
                reduce_g(1, 0)
                reduce_g(3, 1)
                u1 = prod("u", 1, feats[("s", 1)], feats[("c", 1)])
                v1 = prod("v", 1, feats[("s", 1)], feats[("s", 1)])
                C2 = truecos(1, v1, LAMBDA[1])
                reduce_g(5, 2)
                u3 = prod("u", 3, feats[("s", 3)], feats[("c", 3)])
                v3 = prod("v", 3, feats[("s", 3)], feats[("s", 3)])
                C6 = truecos(3, v3, LAMBDA[3])
                reduce_g(7, 3)
                u5 = prod("u", 5, feats[("s", 5)], feats[("c", 5)])
                v5 = prod("v", 5, feats[("s", 5)], feats[("s", 5)])
                reduce_g(9, 4)
                u7 = prod("u", 7, feats[("s", 7)], feats[("c", 7)])
                v7 = prod("v", 7, feats[("s", 7)], feats[("s", 7)])
                reduce_g(11, 5)
                u2 = prod("u", 2, u1, C2)
                v2 = prod("v", 2, u1, u1)
                C4 = truecos(2, v2, LAMBDA[2])
                reduce_g(13, 6)
                u4 = prod("u", 4, u2, C4)
                v4 = prod("v", 4, u2, u2)
                C8 = truecos(4, v4, LAMBDA[4])
                reduce_g(15, 7)
                u6 = prod("u", 6, u3, C6)
                v6 = prod("v", 6, u3, u3)
                u8 = prod("u", 8, u4, C8)
                v8 = prod("v", 8, u4, u4)

                # previous superpass's eviction goes AFTER this superpass's
                # feature emission so ACT's program order is
                # [sp passes][sp+1 passes][evict sp] — eviction fires the
                # moment the PE frees the PSUM, without blocking features.
                if pending is not None:
                    emit_evict(*pending)

                # -- matmuls --
                psum0 = ppool.tile([128, S], FP32, tag="p0")
                psum1 = ppool.tile([128, S], FP32, tag="p1")
                psums = [psum0, psum1]
                n_kt = len(KT_ORDER)
                for kt, fkey in enumerate(KT_ORDER):
                    f = feats[fkey]
                    for oh in range(2):
                        lhsT = wt[:, kt * OUTDIM + oh * 128:
                                  kt * OUTDIM + oh * 128 + 128]
                        for chi in range(S // CH):
                            nc.tensor.matmul(
                                psums[oh][:, chi * CH:(chi + 1) * CH],
                                lhsT, f[:, chi * CH:(chi + 1) * CH],
                                start=(kt == 0), stop=(kt == n_kt - 1),
                            )
                pending = (psums, sp)
            emit_evict(*pending)

        if reps == 1:
            body()
        else:
            with tc.For_i(0, reps, 1):
                body()

    nc.compile()
    _CACHED[key] = nc
    return nc


def _prep_inputs(x: np.ndarray, fouriercoeffs: np.ndarray, bias: np.ndarray):
    xt = np.ascontiguousarray(x.astype(np.float32, copy=False).T)  # (128, 32768)
    C0 = fouriercoeffs[0].astype(np.float32)   # (256, 128, 16) cos coeffs
    C1 = fouriercoeffs[1].astype(np.float32)   # sin coeffs

    # folded weight blocks per kt: w_sb[i, kt*256 + col], col = output o
    w_sb = np.empty((INDIM, 32 * OUTDIM), np.float32)
    for kt, (name, m) in enumerate(KT_ORDER):
        if name == "s":
            blk = C1[:, :, m - 1]                      # (o, i)
        elif name == "c":
            blk = C0[:, :, m - 1]
        elif name == "u":
            blk = KAPPA[m] * C1[:, :, 2 * m - 1]
        else:  # "v"
            blk = -LAMBDA[m] * C0[:, :, 2 * m - 1]
        w_sb[:, kt * OUTDIM:(kt + 1) * OUTDIM] = blk.T
    w_sb = w_sb.astype(np.float16)

    # ACT bias table: 16 passes in order (b_s, b_c) per odd g
    bvals = np.empty(16, np.float32)
    for j, g in enumerate(ODD):
        _, _, _, b_s, b_c = _g_consts(g)
        bvals[2 * j] = b_s
        bvals[2 * j + 1] = b_c
    bt = np.tile(bvals[None, :], (INDIM, 1)).astype(np.float32)

    # folded output bias: bias + sum_i C0[o,i,2m-1] over even harmonics
    bias_fold = bias.reshape(-1).astype(np.float64).copy()
    for m in (1, 2, 3, 4, 5, 6, 7, 8):
        bias_fold += C0[:, :, 2 * m - 1].astype(np.float64).sum(axis=1)
    bias_sb = np.ascontiguousarray(
        bias_fold.astype(np.float32).reshape(2, 128).T)      # (128, 2)
    return xt, w_sb, bt, bias_sb


def kernel(x: np.ndarray, fouriercoeffs: np.ndarray, bias: np.ndarray,
           _trace: bool = False):
    x = np.asarray(x)
    fouriercoeffs = np.asarray(fouriercoeffs)
    bias = np.asarray(bias)
    orig_shape = x.shape
    x2 = x.reshape(-1, INDIM)
    assert x2.shape == (N_TOTAL, INDIM), x2.shape

    nc = _build()
    xt, w_sb, bt, bias_sb = _prep_inputs(x2, fouriercoeffs, bias)
    in_maps = []
    for c in range(N_CORES):
        in_maps.append({
            "xt": np.ascontiguousarray(xt[:, c * N_SHARD:(c + 1) * N_SHARD]),
            "w": w_sb,
            "bt": bt,
            "bias": bias_sb,
        })
    res = run_bass_kernel_spmd(nc, in_maps, list(range(N_CORES)),
                               trace=_trace)
    yt = np.concatenate([res.results[c]["yt"] for c in range(N_CORES)], axis=1)
    y = np.ascontiguousarray(yt.T).astype(np.float32)
    if _trace:
        kernel._last_result = res
    return y.reshape(*orig_shape[:-1], OUTDIM)
